# revision 31
# baseline (speedup 1.0000x reference)
"""Trainium2 Bass kernel for nn_CvxDifflayer (batched PDHG LP solver).

Math (per batch row b):
    u_{k+1} = clip(u_k - tau*(q + y_k @ K), 0, 1)
    ubar    = 2*u_{k+1} - u_k
    y_{k+1} = relu(y_k + sigma*(ubar @ K.T - h))
    out z   = u_300[:, V:]  reshaped (12, 12)

Device reformulation (exact, per 64-batch shard):
    G_k  = tau*(y_k @ K[:, :V])         MM1 (edge cols only): lhsT = y
                                        feat-major bf16, rhs = tau*K bf16
    u_e  = clip(u_e - G_k)              edge cols (q = 0 there), DVE
    u_z  = clip(pres + tau*y2)          z cols on GPSIMD (K z-cols = -I)
    P_k  = sigma*(u @ K.T) - sigma*h    MM2 (full K): lhsT = u feat-major
                                        bf16 via PE transpose, h folded
                                        via constant ones-row
    y    = relu(y + 2*P_k - P_{k-1})    via T3 = 2P + YP, YP' = relu(T3)-P

Software-pipelined across iterations: iteration k's MM1 chunk ci starts
as soon as y-block ci is transposed+relu'd; the y tail of iteration k-1
(T_y1/T_y2/relu1/relu2/YP) fills iteration k's MM1 stream so the PE
never idles (keeps the tensor engine at its top p-state).  Matmul
operands are bf16 (1 cyc/row); states and elementwise math stay fp32.

Layouts per core (batch shard Bs=64):
    U_A   [64, 512]   edge features 0:512 (batch-major, fp32)
    U_B   [64, 644]   edges 512:1012 (cols 0:500, DVE) + z (cols
                      500:644, GPSIMD)
    T3    [64, 288]   pre-relu y;  YFMb = relu(T3^T) bf16 (ACT)
"""

import sys

for _p in ("/opt/trn_rl_repo", "/opt/pypackages"):
    if _p not in sys.path:
        sys.path.insert(0, _p)

import numpy as np

N_GRID = 12
N = 144          # nodes
V = 1012         # directed edges
F = V + N        # 1156 primal vars
YR = 2 * N       # 288 dual vars
B = 512
BS = 64          # batch per core
N_CORES = 8
ITERS = 300
W1 = V           # MM1 rhs width per contraction chunk (edge cols only)
FM_CHUNKS = 10   # ceil(1156/128)
LAST_CW = F - 9 * 128      # 4
FA = 512         # feature split: A = 0:512, B = 512:1156
FB = F - FA      # 644
ZB = FB - N      # 500: z-cols start inside U_B
HH = 144         # MM2 free-dim half (P columns)


def _build_constants(A, A_pos, b):
    K = np.zeros((YR, F), np.float32)
    K[:N, :V] = A
    K[N:, :V] = A_pos
    K[N:, V:] = -np.eye(N, dtype=np.float32)
    h = np.concatenate([b.astype(np.float32), np.zeros(N, np.float32)])
    Kn = np.float32(np.sqrt(np.abs(K).sum(0).max() * np.abs(K).sum(1).max()))
    tau = np.float32(0.9) / Kn
    return K, h, tau


def _host_tiles(K, h, tau):
    """Constant SBUF images shared by all cores (bf16 matmul operands)."""
    import ml_dtypes
    bf16 = ml_dtypes.bfloat16
    sigma = tau
    tauK = (tau * K).astype(np.float32)          # (288, 1156)
    sigK = (sigma * K).astype(np.float32)

    # KA1: MM1 rhs (edge cols only), 3 contraction chunks side by side
    ka1 = np.zeros((128, 3 * W1), np.float32)
    for r in range(3):
        r0 = 128 * r
        rw = min(128, YR - r0)
        ka1[:rw, W1 * r:W1 * r + V] = tauK[r0:r0 + rw, :V]

    # KS2: MM2 rhs, 10 feat chunks of [rows, 288] side by side
    ks2 = np.zeros((128, 10 * YR), np.float32)
    for c in range(FM_CHUNKS):
        cw = 128 if c < 9 else LAST_CW
        f0 = 128 * c
        ks2[:cw, YR * c:YR * c + YR] = sigK[:, f0:f0 + cw].T
        if c == 9:
            ks2[LAST_CW, YR * c:YR * c + YR] = -sigma * h  # ones-row fold
    return ka1.astype(bf16), ks2.astype(bf16)


# constsr layout (bf16, read-only): ka1 | ks2
C_KA1 = 0
C_KS2 = C_KA1 + 3 * W1
CR_W = C_KS2 + 10 * YR
# constsf layout (f32, read-only): tqz (tau*c) | yp0  (yp0 = +sigma*h)
C_TQ = 0
C_YP = C_TQ + N
CF_W = C_YP + YR


def _pack_consts(ka1, ks2, tqz, yp0):
    import ml_dtypes
    bf16 = ml_dtypes.bfloat16
    cr = np.zeros((128, CR_W), bf16)
    cr[:, C_KA1:C_KS2] = ka1
    cr[:, C_KS2:CR_W] = ks2
    cf = np.zeros((64, CF_W), np.float32)
    cf[:, C_TQ:C_YP] = tqz
    cf[:, C_YP:CF_W] = yp0
    return cr, cf


def _build_bass(tau):
    from concourse import bass, mybir
    from concourse.tile import TileContext
    from concourse.tile_rust import add_dep_helper
    from concourse.mybir import AluOpType as op

    f32 = mybir.dt.float32
    bf16 = mybir.dt.bfloat16
    RELU = mybir.ActivationFunctionType.Relu

    nc = bass.Bass()
    d_cr = nc.dram_tensor("constsr", (128, CR_W), bf16, kind="ExternalInput")
    d_cf = nc.dram_tensor("constsf", (64, CF_W), f32, kind="ExternalInput")
    d_z = nc.dram_tensor("z", (64, N), f32, kind="ExternalOutput")

    with TileContext(nc) as tc:
        with (
            tc.tile_pool(name="state", bufs=1) as sp,
            tc.tile_pool(name="psA", bufs=1, space="PSUM") as psA,
            tc.tile_pool(name="psB", bufs=1, space="PSUM") as psB,
            tc.tile_pool(name="psP", bufs=1, space="PSUM") as psP,
            tc.tile_pool(name="psT0", bufs=1, space="PSUM") as psT0,
            tc.tile_pool(name="psT1", bufs=1, space="PSUM") as psT1,
            tc.tile_pool(name="psT2", bufs=1, space="PSUM") as psT2,
            tc.tile_pool(name="psTY", bufs=1, space="PSUM") as psTY,
            tc.tile_pool(name="psTY12", bufs=1, space="PSUM") as psTY12,
        ):
            CONSTR = sp.tile([128, CR_W], bf16)
            CONSTF = sp.tile([64, CF_W], f32)
            KA1 = CONSTR[:, C_KA1:C_KS2]
            KS2 = CONSTR[:, C_KS2:CR_W]
            TQZ = CONSTF[:, C_TQ:C_YP]          # tau*c  [64, 144]
            U_A = sp.tile([64, FA], f32)
            U_B = sp.tile([64, FB], f32)
            TMP_A = sp.tile([64, FA], f32)
            TMP_B = sp.tile([64, ZB], f32)
            TMP_Z = sp.tile([64, N], f32)       # GPSIMD scratch (z cols)
            PRES_T = sp.tile([64, N], f32)      # u_z - tau*c (GPSIMD)
            Y2BM = sp.tile([64, N], f32)        # tau*y2 (ACT relu)
            ZOUT = sp.tile([64, N], f32)        # staged z output
            YPA = sp.tile([64, 128], f32)       # YP cols 0:128
            YPB = sp.tile([64, 160], f32)       # YP cols 128:288
            T3A = sp.tile([64, 128], f32)       # y pre-relu cols 0:128
            T3B = sp.tile([64, 160], f32)       # y pre-relu cols 128:288
            # per-engine scratch tiles (separate so absorber ops never
            # create cross-engine tile deps)
            SCRD = sp.tile([32, 12], f32)
            SCRA = sp.tile([32, 4], f32)
            SCRP = sp.tile([32, 8], f32)
            ONES32 = sp.tile([32, 64], f32)
            ZER128 = sp.tile([128, 192], f32)
            # feat-major u in bf16 tiles; ones-row for the h-fold at
            # row LAST_CW of UFM9
            UFM0 = sp.tile([128, 256], bf16)   # chunks 0..3
            UFM45 = sp.tile([128, 128], bf16)  # chunks 4,5
            UFM67 = sp.tile([128, 128], bf16)  # chunks 6,7
            UFM8 = sp.tile([128, 64], bf16)    # chunk 8
            UFM9 = sp.tile([32, 64], bf16)     # chunk 9 + ones row
            YFM = sp.tile([128, 192], bf16)   # y feat-major, 3 blocks
            IDENT = sp.tile([128, 128], f32)

            dma1 = nc.sync.dma_start(CONSTR[:, :], d_cr[:, :])
            dma2 = nc.sync.dma_start(CONSTF[:, :], d_cf[:, :])

            pool_warm = [
                nc.gpsimd.memset(IDENT[:, :], 0.0),
                nc.gpsimd.affine_select(
                    out=IDENT[:, :], in_=IDENT[:, :],
                    compare_op=mybir.AluOpType.not_equal, fill=1.0, base=0,
                    pattern=[[-1, 128]], channel_multiplier=1),
            ]
            dve_insts = [
                nc.vector.memset(U_A[:, :], 0.0),
                nc.vector.memset(U_B[:, :], 0.0),
                nc.vector.memset(ONES32[:, :], 1.0),
                nc.vector.memset(ZER128[:, :], 0.0),
            ]
            ub_memset = dve_insts[1]

            G_A = psA.tile([64, FA], f32)
            G_B = psB.tile([64, ZB], f32)
            P = psP.tile([64, YR], f32)
            TPX = psT0.tile([128, 256], f32)   # chunks 0..3
            TP47 = psT1.tile([128, 256], f32)  # chunks 4..7
            TP89 = psT2.tile([128, 128], f32)  # chunks 8..9
            TY0 = psTY.tile([128, 64], f32)    # y block 0 pre-relu
            TY12 = psTY12.tile([128, 128], f32)  # y blocks 1,2 pre-relu

            # This target allows only ONE sem wait per instruction. Tile's
            # wait elision relies on per-engine program order, which the
            # scheduler may permute. So: (a) pin every engine's stream to
            # emission order with no_sync edges, (b) warm each engine with
            # ops that absorb foreign sems one at a time, (c) per iteration
            # the emission order is arranged so every real instruction
            # needs at most one new wait (absorber ops where not).
            prev = {}

            def chain(eng, inst, *sync_deps):
                for d in sync_deps:
                    add_dep_helper(inst.ins, d.ins, True, "warm")
                if eng in prev:
                    add_dep_helper(inst.ins, prev[eng].ins, False, "order")
                prev[eng] = inst
                return inst

            def pe(inst, *d):
                return chain("pe", inst, *d)

            def dve(inst, *d):
                return chain("dve", inst, *d)

            def act(inst, *d):
                return chain("act", inst, *d)

            def pool(inst, *d):
                return chain("pool", inst, *d)

            # ---- engine warmups: absorb foreign semaphores one at a time
            dve(nc.vector.tensor_copy(SCRD[0:32, 8:10], CONSTF[0:32, 0:2]),
                dma2)
            dve(nc.vector.tensor_copy(YPA[:, :],
                                      CONSTF[:, C_YP:C_YP + 128]))
            dve(nc.vector.tensor_copy(YPB[:, :],
                                      CONSTF[:, C_YP + 128:CF_W]))
            pool(nc.gpsimd.tensor_copy(SCRP[0:32, 4:8], CONSTF[0:32, 4:8]),
                 dma2)
            pool(nc.gpsimd.tensor_scalar_mul(PRES_T[:, :], TQZ, -1.0))
            # pool absorber for the dve U_B memset (iter-0 z-clip WAR);
            # source is dve-written so both deps merge onto the DVE sem
            pool(nc.gpsimd.tensor_copy(SCRP[0:32, 0:4], ZER128[0:32, 4:8]),
                 ub_memset)
            act(nc.scalar.copy(UFM9[0:32, 0:64], ONES32[:, :]),
                *dve_insts)
            act(nc.scalar.copy(Y2BM[:, :], ZER128[0:64, 0:144]),
                *pool_warm)
            yfm_init = act(nc.scalar.copy(YFM[:, :], ZER128[:, 0:192]))
            # absorb the unchained dve memsets' sem on DVE itself (later
            # same-engine deps on them then elide as covered)
            dve(nc.vector.tensor_copy(SCRD[0:32, 10:12], ZER128[0:32, 0:2]))
            pe(nc.tensor.transpose(G_A[0:64, 0:64], IDENT[0:64, 0:64],
                                   IDENT[0:64, 0:64]),
               *pool_warm)
            pe(nc.tensor.transpose(G_A[0:64, 0:64], U_A[:, 0:64],
                                   IDENT[0:64, 0:64]),
               *dve_insts)
            pe(nc.tensor.matmul(G_A[0:64, 0:64], KS2[0:128, 0:64],
                                KA1[0:128, 0:64], start=True, stop=True),
               dma1)

            def mm1(dst, col0, width, ci, start, stop):
                rw = 128 if ci < 2 else 32
                return pe(nc.tensor.matmul(
                    dst[:, 0:width], YFM[0:rw, 64 * ci:64 * ci + 64],
                    KA1[0:rw, W1 * ci + col0:W1 * ci + col0 + width],
                    start=start, stop=stop))

            def t_u(c):
                # transpose u feature chunk c to feat-major (PSUM)
                if c < 4:
                    return pe(nc.tensor.transpose(
                        TPX[:, 64 * c:64 * c + 64],
                        U_A[:, 128 * c:128 * c + 128], IDENT[0:64, 0:64]))
                if c < 8:
                    return pe(nc.tensor.transpose(
                        TP47[0:128, 64 * (c - 4):64 * (c - 4) + 64],
                        U_B[:, 128 * (c - 4):128 * (c - 4) + 128],
                        IDENT[0:64, 0:64]))
                cw = 128 if c < 9 else LAST_CW
                return pe(nc.tensor.transpose(
                    TP89[0:cw, 64 * (c - 8):64 * (c - 8) + 64],
                    U_B[:, 128 * (c - 4):128 * (c - 4) + cw],
                    IDENT[0:64, 0:64]))

            UFMT = {0: (UFM0, 0), 1: (UFM0, 64), 2: (UFM0, 128),
                    3: (UFM0, 192), 4: (UFM45, 0), 5: (UFM45, 64),
                    6: (UFM67, 0), 7: (UFM67, 64), 8: (UFM8, 0),
                    9: (UFM9, 0)}

            def mm2(c, h0, h1, start=False, stop=False):
                rows = 128 if c < 9 else LAST_CW + 1
                tile, col = UFMT[c]
                return pe(nc.tensor.matmul(
                    P[:, h0:h1], tile[0:rows, col:col + 64],
                    KS2[0:rows, YR * c + h0:YR * c + h1],
                    start=start, stop=stop, skip_group_check=True))

            for it in range(ITERS):
                first = (it == 0)

                # absorbers: abs_a picks up the ACT sem at Y2BM's write
                # (covers T3B's WAR vs its act read); abs_P1 picks up the
                # PE sem at P-half1's close (covers T3b/YPa/YPb P reads)
                dve(nc.vector.tensor_copy(SCRD[0:32, 0:2], Y2BM[0:32, 0:2]))
                # abs_b: advance our own sem past prev body's T3a so
                # T3b's P reader-chain dep is pre-covered
                dve(nc.vector.tensor_copy(SCRD[0:32, 2:4], SCRD[0:32, 2:4]),
                    *( [prev_t3a] if not first else [] ))
                # === prev-iter y tail: T3b on DVE (needs P half1) ===
                if not first:
                    dve(nc.vector.scalar_tensor_tensor(
                        T3B[:, :], P[:, 128:YR], 2.0, YPB[:, :],
                        op.mult, op.add))

                # === MM1 chunk 0 (y rows 0:128) ===
                mm1(G_A, 0, FA, 0, True, False)
                if not first:
                    pe(nc.tensor.transpose(TY12[0:128, 0:64],
                                           T3B[:, 0:128],
                                           IDENT[0:64, 0:64]))
                    pe(nc.tensor.transpose(TY12[0:32, 64:128],
                                           T3B[:, 128:160],
                                           IDENT[0:64, 0:64]))
                    # one relu for y blocks 1+2 (rows 32:128 of block 2
                    # are garbage but unread by MM1)
                    act(nc.scalar.activation(YFM[0:128, 64:192],
                                             TY12[0:128, 0:128], RELU))
                    # absorber: pick up prev iter's GPSIMD TMP_Z write so
                    # Y2BM's pool WAR is pre-covered
                    act(nc.scalar.copy(SCRA[0:32, 0:2], TMP_Z[0:32, 0:2]))
                    # Y2BM = tau*y2 = relu(tau*T3[:,144:288]) feeds the
                    # GPSIMD z-column update
                    act(nc.scalar.activation(Y2BM[:, :], T3B[:, 16:160],
                                             RELU, scale=float(tau)))
                    # YP' = relu(T3) - P (prev iter, off critical path)
                    dve(nc.vector.scalar_tensor_tensor(
                        YPA[:, :], T3A[:, :], 0.0, P[:, 0:128],
                        op.max, op.subtract))
                    dve(nc.vector.scalar_tensor_tensor(
                        YPB[:, :], T3B[:, :], 0.0, P[:, 128:YR],
                        op.max, op.subtract))

                # === GPSIMD z-column update (u_z = clip(pres+tau*y2)) ===
                # abs1: self-wait on prev-iter sub; then each op carries
                # exactly one new wait.
                pool(nc.gpsimd.tensor_copy(SCRP[0:32, 0:2],
                                           PRES_T[0:32, 0:2]))
                pool(nc.gpsimd.tensor_add(TMP_Z[:, :], Y2BM[:, :],
                                          PRES_T[:, :]))
                # abs2: absorb PE sem (prev-iter t_u(7) is the LAST
                # reader of U_B z cols; covers t8/t9 too). Source is
                # dma-backed so no pool-self data dep.
                pool(nc.gpsimd.tensor_copy(SCRP[0:32, 2:4],
                                           CONSTF[0:32, 8:10]),
                     *( [prev_t7] if not first else [] ))
                pool(nc.gpsimd.tensor_scalar(
                    U_B[:, ZB:FB], TMP_Z[:, :], 0.0, 1.0, op.max, op.min))
                pool(nc.gpsimd.tensor_sub(PRES_T[:, :], U_B[:, ZB:FB],
                                          TQZ))

                # === MM1 chunks 1,2 + B half ===
                mm1(G_A, 0, FA, 1, False, False)
                mm1(G_A, 0, FA, 2, False, True)
                mm1(G_B, FA, ZB, 0, True, False)
                mm1(G_B, FA, ZB, 1, False, False)
                mm1(G_B, FA, ZB, 2, False, True)

                # === u update on DVE (edge cols; pres = u since q = 0) ===
                dve(nc.vector.scalar_tensor_tensor(
                    TMP_A[:, :], G_A[:, :], -1.0, U_A[:, :],
                    op.mult, op.add))
                dve(nc.vector.tensor_scalar(
                    U_A[:, :], TMP_A[:, :], 0.0, 1.0, op.max, op.min))
                dve(nc.vector.scalar_tensor_tensor(
                    TMP_B[:, :], G_B[:, :], -1.0, U_B[:, 0:ZB],
                    op.mult, op.add))
                dve(nc.vector.tensor_scalar(
                    U_B[:, 0:ZB], TMP_B[:, :], 0.0, 1.0, op.max, op.min))

                # === transpose u, MM2 accumulates P ===
                for c in range(4):
                    t_u(c)
                act(nc.scalar.copy(UFM0[:, 0:128], TPX[:, 0:128]))
                act(nc.scalar.copy(UFM0[:, 128:256], TPX[:, 128:256]))
                mm2(0, 0, YR, start=True)
                mm2(1, 0, YR)
                # z chunks first: their only writer is the GPSIMD clip,
                # so t_u(8)/t_u(9) carry the single pool wait
                t_u(8)
                t_u(9)
                act(nc.scalar.copy(UFM8[:, :], TP89[:, 0:64]))
                act(nc.scalar.copy(UFM9[0:LAST_CW, 0:64],
                                   TP89[0:LAST_CW, 64:128]))
                mm2(2, 0, YR)
                mm2(3, 0, YR)
                t_u(4)
                t_u(5)
                t_u(6)
                prev_t7 = t_u(7)
                act(nc.scalar.copy(UFM45[:, :], TP47[:, 0:128]))
                act(nc.scalar.copy(UFM67[:, :], TP47[:, 128:256]))
                mm2(8, 0, YR)
                mm2(9, 0, YR)
                mm2(4, 0, YR)
                mm2(5, 0, YR)
                mm2(6, 0, HH)
                mm2(7, 0, HH, stop=True)      # close P half0
                # === y tail for this iter: T3a, T_y0 while h1 streams ===
                prev_t3a = dve(nc.vector.scalar_tensor_tensor(
                    T3A[:, :], P[:, 0:128], 2.0, YPA[:, :],
                    op.mult, op.add))
                mm2(6, HH, YR)
                mm2(7, HH, YR, stop=True)     # close P half1
                pe(nc.tensor.transpose(TY0[0:128, 0:64], T3A[:, :],
                                       IDENT[0:64, 0:64]))
                act(nc.scalar.activation(YFM[0:128, 0:64],
                                         TY0[0:128, 0:64], RELU))

            # stage z into a fresh tile (single chained writer) so the
            # output DMA carries exactly one wait
            dve(nc.vector.tensor_copy(ZOUT[:, :], U_B[:, ZB:FB]))
            zdma = nc.sync.dma_start(d_z[:, :], ZOUT[:, :])
            # tail fence: the framework drain waits on every proc, but the
            # ISA allows one wait per instruction — absorb them one at a
            # time with SP nops so the drain's own waits are elided.
            for d in (dma1, dma2, prev["pool"], prev["act"], prev["pe"],
                      prev["dve"], zdma):
                nn = nc.sync.nop()
                add_dep_helper(nn.ins, d.ins, True, "tail fence")
    return nc


LAST_RESULT = None


def kernel(weights, A, A_pos, b, _trace=False):
    weights = np.asarray(weights, np.float32)
    A = np.asarray(A, np.float32)
    A_pos = np.asarray(A_pos, np.float32)
    b = np.asarray(b, np.float32)

    K, h, tau = _build_constants(A, A_pos, b)
    ka1, ks2 = _host_tiles(K, h, tau)
    yp0 = np.broadcast_to(tau * h, (64, YR)).astype(np.float32).copy()

    nc = _build_bass(tau)

    in_maps = []
    for core in range(N_CORES):
        w_shard = weights[core * BS:(core + 1) * BS].reshape(BS, N)
        tqz = (tau * w_shard).astype(np.float32)
        cr, cf = _pack_consts(ka1, ks2, tqz, yp0)
        in_maps.append({"constsr": cr, "constsf": cf})

    from concourse.bass_utils import run_bass_kernel_spmd
    res = run_bass_kernel_spmd(nc, in_maps, core_ids=list(range(N_CORES)),
                               trace=_trace)
    global LAST_RESULT
    LAST_RESULT = res
    outs = [np.asarray(res.results[c]["z"]) for c in range(N_CORES)]
    z = np.concatenate(outs, axis=0).reshape(B, N_GRID, N_GRID)
    return z.astype(np.float32)


if __name__ == "__main__":
    # smoke build only
    _build_bass(np.float32(0.13))
    print("bass build OK")
